# revision 40
# baseline (speedup 1.0000x reference)
"""DeepSeek-V3 MLA attention on 8 TRN2 NeuronCores (Bass/Tile).

Self-contained kernel: kernel(**inputs) takes the full unsharded inputs and
returns the full [2, 2048, 2048] float32 output.

Sharding: row-data-parallel projections (each core owns 512 of the 4096
token rows, computes all 16 heads), AllToAll into head-sharded attention
(2 heads per core over all rows), AllToAll back to row-parallel for the
output projection. Four collectives total (measured A2A latency here is
~30us nearly independent of size, so fewer+bigger wins): one KV (K fp8 +
K residual + k_pe bf16 + V bf16, issued right after the early KV-latent
pass so it overlaps the Q side), one Q (nope fp8 + rope bf16), and one
output A2A per head-pair so the first half of the output projection
overlaps the second head-pair's attention.

Attention scores run as fp8 DoubleRow matmuls over (K, K-residual) pairs
-- the residual subtile makes K effectively exact, leaving only the fp8
noise of Q-nope -- plus a bf16 rope matmul (rope carries ~2/3 of the
score variance, so it stays high precision). The softmax scale is split
192^-1/4 onto each side. Softmax denominators accumulate on the PE via a
ones-vector matmul chain in PSUM; exp runs on the activation engine over
fused 2-block tiles. V, probabilities, and the output path stay bf16.
"""
from contextlib import ExitStack

import numpy as np
import ml_dtypes

import concourse.bass as bass
import concourse.mybir as mybir
import concourse.tile as tile
from concourse import bacc
from concourse.bass_utils import run_bass_kernel_spmd

BF16NP = ml_dtypes.bfloat16

dt = mybir.dt
F32, BF16, FP8 = dt.float32, dt.bfloat16, dt.float8e4
DR = mybir.MatmulPerfMode.DoubleRow

P = 128
NC_ = 8
LR = 512               # local rows per core
NH = 16
Q_LORA, KV_LORA = 1536, 512
NOPE, ROPE, VH = 128, 64, 128
EPS = 1e-6
B, S = 2, 2048
R = B * S
S4 = float(192.0 ** -0.25)   # half of the softmax scale on each of q/k

# KV A2A shard (bytes): k8 | kr8 residual | kpe bf16 | V bf16 blocks
KV_KB = 0                      # 2 heads of [128, 512] fp8
KV_RB = 2 * 128 * 512          # 2 heads of [128, 512] fp8 (residual)
KV_PB = 4 * 128 * 512          # [64, 512] bf16
KV_VB = KV_PB + 64 * 512 * 2   # 8 (rt, hl) blocks of [128, 128] bf16
SHKV = KV_VB + 512 * 256 * 2
# Q A2A shard (bytes): nope fp8 [128,512] x2 heads | rope bf16 [64,512] x2
Q_NE = 0
Q_RE = 2 * 128 * 512
SHQ = Q_RE + 2 * 64 * 512 * 2
SH2 = 128 * 512


def build_kernel(reps: int = 1, debug: bool = False, loopback: bool = False):
    nc = bacc.Bacc(None, target_bir_lowering=False, debug=False)

    XT = nc.dram_tensor("xt", [2048, LR], BF16, kind="ExternalInput")
    WAKV = nc.dram_tensor("wakv", [2048, 576], BF16, kind="ExternalInput")
    WAQ = nc.dram_tensor("waq", [2048, Q_LORA], BF16, kind="ExternalInput")
    WQB = nc.dram_tensor("wqb", [Q_LORA, NH * 192], BF16, kind="ExternalInput")
    WKVB = nc.dram_tensor("wkvb", [KV_LORA, NH * 256], BF16, kind="ExternalInput")
    WO = nc.dram_tensor("wo", [2048, 2048], BF16, kind="ExternalInput")
    COST = nc.dram_tensor("cost", [P, LR], F32, kind="ExternalInput")
    SINT = nc.dram_tensor("sint", [P, LR], F32, kind="ExternalInput")
    OUT = nc.dram_tensor("out", [LR, 2048], F32, kind="ExternalOutput")

    SENDKV = nc.dram_tensor("sendkv", [NC_, SHKV], dt.uint8, kind="Internal")
    RECVKV = nc.dram_tensor("recvkv", [NC_, SHKV], dt.uint8, kind="Internal")
    SENDQ = nc.dram_tensor("sendq", [NC_, SHQ], dt.uint8, kind="Internal")
    RECVQ = nc.dram_tensor("recvq", [NC_, SHQ], dt.uint8, kind="Internal")
    SEND2 = [nc.dram_tensor(f"send2{h}", [NC_, SH2], BF16, kind="Internal")
             for h in range(2)]
    RECV2 = [nc.dram_tensor(f"recv2{h}", [NC_, SH2], BF16, kind="Internal")
             for h in range(2)]

    with tile.TileContext(nc) as tc, ExitStack() as octx:
        consts = octx.enter_context(tc.tile_pool(name="consts", bufs=1))
        ones_bf = consts.tile([P, 1], BF16)
        nc.vector.memset(ones_bf, 1.0)
        masks = consts.tile([P, 4, 512], BF16)
        for m in range(4):
            nc.gpsimd.memset(masks[:, m, :], 1.0)
            # keep where qf - kp - 128m >= 0 else 0
            nc.gpsimd.affine_select(
                out=masks[:, m, :], in_=masks[:, m, :],
                compare_op=mybir.AluOpType.is_ge, fill=0.0,
                base=-128 * m, pattern=[[1, 512]], channel_multiplier=-1,
            )
        eps_t = consts.tile([1, 1], F32)
        nc.vector.memset(eps_t, EPS)
        cos_sb = consts.tile([P, LR], F32)
        sin_sb = consts.tile([P, LR], F32)
        nc.sync.dma_start(out=cos_sb, in_=COST[:, :])
        nc.sync.dma_start(out=sin_sb, in_=SINT[:, :])
        cst = dict(ones_bf=ones_bf, masks=masks, cos=cos_sb, sin=sin_sb,
                   eps=eps_t)

        for rep in range(reps):
            _one_rep(nc, tc, rep, XT, WAKV, WAQ, WQB, WKVB, WO, OUT,
                     SENDKV, RECVKV, SENDQ, RECVQ, SEND2, RECV2, cst, loopback)
    nc.finalize()
    return nc


def _bv(dram, j, byte_off, rows, width, dtype):
    """[rows, width] typed view at byte offset of uint8 shard j."""
    esz = {BF16: 2, FP8: 1, F32: 4}[dtype]
    sl = dram[j, byte_off:byte_off + rows * width * esz]
    return sl.rearrange("(p c) -> p c", c=width * esz).bitcast(dtype)


def _one_rep(nc, tc, rep, XT, WAKV, WAQ, WQB, WKVB, WO, OUT,
             SENDKV, RECVKV, SENDQ, RECVQ, SEND2, RECV2, cst, loopback=False):
    cos_sb, sin_sb = cst["cos"], cst["sin"]
    ones_bf, masks, eps_t = cst["ones_bf"], cst["masks"], cst["eps"]

    def _a2a(send, recv):
        if loopback:
            nc.sync.dma_start(out=recv, in_=send)
        else:
            nc.gpsimd.collective_compute(
                "AllToAll", mybir.AluOpType.bypass,
                ins=[send], outs=[recv],
                replica_groups=[list(range(NC_))])

    with ExitStack() as ctx:
      with ExitStack() as pctx:
        # inputs needed through stage C/D
        xtp = pctx.enter_context(tc.tile_pool(name=f"xt{rep}", bufs=1))
        xt_sb = xtp.tile([P, 16, LR], BF16)
        lats = pctx.enter_context(tc.tile_pool(name=f"lats{rep}", bufs=1))
        latkv = lats.tile([P, 4, 512], BF16)     # normed kv latent^T
        latkp = lats.tile([64, 512], BF16)       # raw rope latent^T
        kpeB = lats.tile([64, 512], BF16)        # roped k_pe^T
        latq = lats.tile([P, 12, 512], BF16)     # normed q latent^T
        vkp = pctx.enter_context(tc.tile_pool(name=f"vk{rep}", bufs=1))
        v_sb = vkp.tile([P, 4, 2048], BF16)      # V token-major
        kt8 = vkp.tile([P, 16, 512], FP8)        # K^T nope per head (fp8)
        kr8 = vkp.tile([P, 16, 512], FP8)        # fp8 residual of K^T nope

        xt_v = XT[:, :].rearrange("(kt p) n -> p kt n", p=P)
        wakv_v = WAKV[:, :].rearrange("(kt p) n -> p kt n", p=P)
        waq_v = WAQ[:, :].rearrange("(kt p) n -> p kt n", p=P)
        wkvb_v = WKVB[:, :].rearrange("(kt p) n -> p kt n", p=P)

        # ---------------- Stage A: kv latent + rmsnorm + kpe rope ----------
        with ExitStack() as sctx:
            wakvp = sctx.enter_context(tc.tile_pool(name=f"wakv{rep}", bufs=1))
            wkvbp = sctx.enter_context(tc.tile_pool(name=f"wkvb{rep}", bufs=1))
            waqp = sctx.enter_context(tc.tile_pool(name=f"waq{rep}", bufs=1))
            ppA = sctx.enter_context(tc.tile_pool(name=f"psA{rep}", bufs=3, space="PSUM"))
            ppq = sctx.enter_context(tc.tile_pool(name=f"psq{rep}", bufs=1, space="PSUM"))
            nrm = sctx.enter_context(tc.tile_pool(name=f"nrm{rep}", bufs=1))

            wakv_sb = wakvp.tile([P, 16, 576], BF16)
            wkvb_sb = wkvbp.tile([P, 4, NH * 256], BF16)
            waq_sb = waqp.tile([P, 16, Q_LORA], BF16)
            for kt in range(0, 16, 4):
                nc.sync.dma_start(out=xt_sb[:, kt:kt + 4, :],
                                  in_=xt_v[:, kt:kt + 4, :])
                nc.sync.dma_start(out=wakv_sb[:, kt:kt + 4, :],
                                  in_=wakv_v[:, kt:kt + 4, :])
            nc.sync.dma_start(out=wkvb_sb[:, :, :], in_=wkvb_v[:, :, :])
            for kt in range(0, 16, 8):
                nc.sync.dma_start(out=waq_sb[:, kt:kt + 8, :],
                                  in_=waq_v[:, kt:kt + 8, :])

            sq_kv = nrm.tile([P, 4, 512], BF16)
            for pt in range(4):
                ps = ppA.tile([P, 512], F32, tag="psA")
                for kt in range(16):
                    nc.tensor.matmul(
                        ps, lhsT=wakv_sb[:, kt, pt * 128:(pt + 1) * 128],
                        rhs=xt_sb[:, kt, :], start=(kt == 0), stop=(kt == 15))
                nc.scalar.copy(latkv[:, pt, :], ps)
                nc.scalar.square(sq_kv[:, pt, :], ps)
            ps_kp = ppA.tile([64, 512], F32, tag="psA")
            for kt in range(16):
                nc.tensor.matmul(ps_kp, lhsT=wakv_sb[:, kt, 512:576],
                                 rhs=xt_sb[:, kt, :], start=(kt == 0), stop=(kt == 15))
            nc.scalar.copy(latkp, ps_kp)

            ps_ssq = ppq.tile([1, 512], F32, tag="psq")
            for pt in range(4):
                nc.tensor.matmul(ps_ssq, lhsT=ones_bf, rhs=sq_kv[:, pt, :],
                                 start=(pt == 0), stop=(pt == 3))
            rkv = nrm.tile([1, 512], F32)
            nc.scalar.activation(rkv, ps_ssq, mybir.ActivationFunctionType.Sqrt,
                                 bias=eps_t, scale=1.0 / KV_LORA)
            nc.vector.reciprocal(rkv, rkv)
            rkv_b = nrm.tile([P, 512], F32)
            nc.gpsimd.partition_broadcast(rkv_b, rkv)
            for pt in range(4):
                nc.vector.tensor_mul(latkv[:, pt, :], latkv[:, pt, :], rkv_b)

            # kpe rope (unnormed): rows [e(32)|o(32)]
            kp = nrm.tile([32, 4, 512], F32, tag="krope")
            xo_c = nrm.tile([32, 512], BF16, tag="kxo")
            nc.sync.dma_start(out=xo_c, in_=latkp[32:64, :])
            xe = latkp[0:32, :]
            c32, s32 = cos_sb[0:32, :], sin_sb[0:32, :]
            nc.vector.tensor_mul(kp[:, 0, :], xe, c32)
            nc.vector.tensor_mul(kp[:, 1, :], xe, s32)
            nc.vector.tensor_mul(kp[:, 2, :], xo_c, s32)
            nc.vector.tensor_mul(kp[:, 3, :], xo_c, c32)
            nc.vector.tensor_sub(kpeB[0:32, :], kp[:, 0, :], kp[:, 2, :])
            yiB = nrm.tile([32, 512], BF16, tag="kyi")
            nc.vector.tensor_add(yiB, kp[:, 1, :], kp[:, 3, :])
            nc.sync.dma_start(out=kpeB[32:64, :], in_=yiB)

            # two q-latent chains interleaved here to hide the kv-norm tail
            sq_q = nrm.tile([P, 12, 512], BF16, tag="sqq")

            def _qlat_chain(pt):
                ps = ppA.tile([P, 512], F32, tag="psA")
                for kt in range(16):
                    nc.tensor.matmul(
                        ps, lhsT=waq_sb[:, kt, pt * 128:(pt + 1) * 128],
                        rhs=xt_sb[:, kt, :], start=(kt == 0), stop=(kt == 15))
                nc.scalar.copy(latq[:, pt, :], ps)
                nc.scalar.square(sq_q[:, pt, :], ps)

            # ---------------- Stage B: K first (early A2A), then V ---------
            ppB = sctx.enter_context(tc.tile_pool(name=f"psB{rep}", bufs=4, space="PSUM"))
            for h in range(NH):
                ps = ppB.tile([P, 512], F32, tag="psB")
                for kt in range(4):
                    nc.tensor.matmul(
                        ps, lhsT=wkvb_sb[:, kt, h * 256:h * 256 + 128],
                        rhs=latkv[:, kt, :], start=(kt == 0), stop=(kt == 3))
                nc.scalar.copy(kt8[:, h, :], ps)
                nc.vector.tensor_sub(kr8[:, h, :], ps, kt8[:, h, :])
            for j in range(NC_):
                for hl in range(2):
                    nc.sync.dma_start(
                        out=_bv(SENDKV, j, KV_KB + hl * 65536, 128, 512, FP8),
                        in_=kt8[:, 2 * j + hl, :])
                    nc.sync.dma_start(
                        out=_bv(SENDKV, j, KV_RB + hl * 65536, 128, 512, FP8),
                        in_=kr8[:, 2 * j + hl, :])
                nc.sync.dma_start(out=_bv(SENDKV, j, KV_PB, 64, 512, BF16),
                                  in_=kpeB)

            wkvb_g = wkvb_sb.rearrange("p kt (h two vh) -> p kt h two vh",
                                       two=2, vh=128)
            for g in range(4):
                for rt in range(4):
                    ps = ppB.tile([P, 512], F32, tag="psB")
                    rhs = wkvb_g[:, :, 4 * g:4 * g + 4, 1, :]
                    for kt in range(4):
                        nc.tensor.matmul(
                            ps, lhsT=latkv[:, kt, rt * 128:(rt + 1) * 128],
                            rhs=rhs[:, kt, :, :], start=(kt == 0), stop=(kt == 3))
                    nc.scalar.copy(v_sb[:, rt, g * 512:(g + 1) * 512], ps)
            for j in range(NC_):
                for rt in range(4):
                    for hl in range(2):
                        off = KV_VB + (rt * 2 + hl) * 32768
                        nc.sync.dma_start(
                            out=_bv(SENDKV, j, off, 128, 128, BF16),
                            in_=v_sb[:, rt, 256 * j + hl * 128:
                                     256 * j + (hl + 1) * 128])
            _a2a(SENDKV[:, :], RECVKV[:, :])

            # ---------------- Stage C: q latent + rmsnorm ------------------
            for pt in range(12):
                _qlat_chain(pt)
            ps_ssq_q = ppq.tile([1, 512], F32, tag="psq")
            for pt in range(12):
                nc.tensor.matmul(ps_ssq_q, lhsT=ones_bf, rhs=sq_q[:, pt, :],
                                 start=(pt == 0), stop=(pt == 11))
            rq = nrm.tile([1, 512], F32, tag="rq")
            nc.scalar.activation(rq, ps_ssq_q, mybir.ActivationFunctionType.Sqrt,
                                 bias=eps_t, scale=1.0 / Q_LORA)
            nc.vector.reciprocal(rq, rq)
            rq_b = nrm.tile([P, 512], F32, tag="rqb")
            nc.gpsimd.partition_broadcast(rq_b, rq)
            for pt in range(12):
                nc.vector.tensor_mul(latq[:, pt, :], latq[:, pt, :], rq_b)

        # ---------------- Stage D: Q projection + rope, Q A2As -------------
        with ExitStack() as sctx:
            wqbp = sctx.enter_context(tc.tile_pool(name=f"wqb{rep}", bufs=1))
            ppD = sctx.enter_context(tc.tile_pool(name=f"psD{rep}", bufs=4, space="PSUM"))
            qtp = sctx.enter_context(tc.tile_pool(name=f"qt{rep}", bufs=1))
            rp = sctx.enter_context(tc.tile_pool(name=f"qrope{rep}", bufs=2))

            # per-column-block loads so chains start as soon as each arrives
            wqb_sb = wqbp.tile([P, 12, NH * 192], BF16)
            wqb_c = WQB[:, :].rearrange("(kt p) (pt c) -> p kt pt c", p=P, c=128)
            wqb_s = wqb_sb.rearrange("p kt (pt c) -> p kt pt c", c=128)
            for pt in (16, 20, 0, 4, 8, 12):
                nc.sync.dma_start(out=wqb_s[:, :, pt:pt + 4, :],
                                  in_=wqb_c[:, :, pt:pt + 4, :])

            qt8n = qtp.tile([P, 16, 512], FP8)    # nope per head (fp8)
            qtR = qtp.tile([P, 8, 512], BF16)     # rope e/o tiles (pre-rope)
            qrB = qtp.tile([P, 8, 512], BF16)     # roped e/o tiles

            def _qchain(pt):
                ps = ppD.tile([P, 512], F32)
                for kt in range(12):
                    nc.tensor.matmul(
                        ps, lhsT=wqb_s[:, kt, pt, :],
                        rhs=latq[:, kt, :], start=(kt == 0), stop=(kt == 11))
                if pt < 16:
                    nc.scalar.copy(qt8n[:, pt, :], ps)
                else:
                    nc.scalar.copy(qtR[:, pt - 16, :], ps)

            # rope chains first: rope math (DVE) overlaps the nope chains
            for pt in range(16, 24):
                _qchain(pt)
            for j in range(4):
                et = qtR[:, j, :]
                ot = qtR[:, 4 + j, :]
                t = rp.tile([P, 4, 512], F32, tag="qr")
                nc.vector.tensor_mul(t[:, 0, :], et, cos_sb)
                nc.vector.tensor_mul(t[:, 1, :], et, sin_sb)
                nc.vector.tensor_mul(t[:, 2, :], ot, sin_sb)
                nc.vector.tensor_mul(t[:, 3, :], ot, cos_sb)
                nc.vector.tensor_sub(qrB[:, j, :], t[:, 0, :], t[:, 2, :])
                nc.vector.tensor_add(qrB[:, 4 + j, :], t[:, 1, :], t[:, 3, :])

            for pt in range(16):
                _qchain(pt)
            for hl in range(2):
                for j in range(NC_):
                    h = 2 * j + hl
                    pe = (h % 4) * 32
                    nc.sync.dma_start(
                        out=_bv(SENDQ, j, Q_NE + hl * 65536, 128, 512, FP8),
                        in_=qt8n[:, h, :])
                    nc.sync.dma_start(
                        out=_bv(SENDQ, j, Q_RE + hl * 65536, 32, 512, BF16),
                        in_=qrB[pe:pe + 32, h // 4, :])
                    nc.sync.dma_start(
                        out=_bv(SENDQ, j, Q_RE + hl * 65536 + 32768, 32, 512, BF16),
                        in_=qrB[pe:pe + 32, 4 + h // 4, :])
            _a2a(SENDQ[:, :], RECVQ[:, :])
      # projection pools freed here
      if True:
        # ---------------- Stage E: attention + split output proj -----------
        wop = ctx.enter_context(tc.tile_pool(name=f"wo{rep}", bufs=1))
        wo_sb = wop.tile([P, 16, 2048], BF16)
        wo_v = WO[:, :].rearrange("(kt p) n -> p kt n", p=P)
        with tc.tile_wait_until(0.2):
            for kt in range(0, 16, 8):
                nc.sync.dma_start(out=wo_sb[:, kt:kt + 8, :],
                                  in_=wo_v[:, kt:kt + 8, :])
        otf = wop.tile([P, 16, 512], BF16)
        acc6 = wop.tile([P, 4, 2048], F32)

        with ExitStack() as sctx:
            asm = sctx.enter_context(tc.tile_pool(name=f"asm{rep}", bufs=2))
            ptp = sctx.enter_context(tc.tile_pool(name=f"pt{rep}", bufs=4))
            ppS = sctx.enter_context(tc.tile_pool(name=f"psS{rep}", bufs=2, space="PSUM"))
            ppO = sctx.enter_context(tc.tile_pool(name=f"psO{rep}", bufs=2, space="PSUM"))
            ppN = sctx.enter_context(tc.tile_pool(name=f"psN{rep}", bufs=1, space="PSUM"))
            pp6 = sctx.enter_context(tc.tile_pool(name=f"ps6{rep}", bufs=1, space="PSUM"))
            sml = sctx.enter_context(tc.tile_pool(name=f"sml{rep}", bufs=2))
            otp = sctx.enter_context(tc.tile_pool(name=f"ot{rep}", bufs=1))
            Exp = mybir.ActivationFunctionType.Exp

            for hl in range(2):
                ot_sb = otp.tile([P, 4096], BF16, tag=f"ot{hl}")
                for b in range(B):
                    ktn2 = asm.tile([P, 4, 2, 512], FP8, tag="ktn2")
                    kpeT = asm.tile([64, 4, 512], BF16, tag="kpeT")
                    qt = asm.tile([P, 4, 2, 512], FP8, tag="qt")
                    qpeT = asm.tile([64, 4, 512], BF16, tag="qpeT")
                    vt = asm.tile([P, 16, 128], BF16, tag="vt")
                    for i in range(4):
                        src_ = 4 * b + i
                        nc.sync.dma_start(
                            out=ktn2[:, i, 0, :],
                            in_=_bv(RECVKV, src_, KV_KB + hl * 65536, 128, 512, FP8))
                        nc.sync.dma_start(
                            out=ktn2[:, i, 1, :],
                            in_=_bv(RECVKV, src_, KV_RB + hl * 65536, 128, 512, FP8))
                        nc.sync.dma_start(
                            out=kpeT[:, i, :],
                            in_=_bv(RECVKV, src_, KV_PB, 64, 512, BF16))
                        qn_v = _bv(RECVQ, src_, Q_NE + hl * 65536, 128, 512, FP8)
                        nc.sync.dma_start(out=qt[:, i, 0, :], in_=qn_v)
                        nc.sync.dma_start(out=qt[:, i, 1, :], in_=qn_v)
                        nc.sync.dma_start(
                            out=qpeT[:, i, :],
                            in_=_bv(RECVQ, src_, Q_RE + hl * 65536, 64, 512, BF16))
                        for rt in range(4):
                            off = KV_VB + (rt * 2 + hl) * 32768
                            nc.sync.dma_start(
                                out=vt[:, 4 * i + rt, :],
                                in_=_bv(RECVKV, src_, off, 128, 128, BF16))
                    for qg in range(4):
                        psO = ppO.tile([P, 512], F32)
                        psN = ppN.tile([1, 512], F32)
                        nkt = 4 * qg + 4
                        nch = nkt // 2
                        for ch in range(nch):
                            psS = ppS.tile([P, 2, 512], F32, tag="psS")
                            for u in range(2):
                                kt = 2 * ch + u
                                ks = slice((kt % 4) * 128, (kt % 4 + 1) * 128)
                                nc.tensor.matmul(
                                    psS[:, u, :],
                                    lhsT=ktn2[:, kt // 4, :, ks],
                                    rhs=qt[:, qg, :, :],
                                    start=True, stop=False, perf_mode=DR)
                                nc.tensor.matmul(
                                    psS[:, u, :],
                                    lhsT=kpeT[:, kt // 4, ks],
                                    rhs=qpeT[:, qg, :],
                                    start=False, stop=True)
                            pt2 = ptp.tile([P, 2, 512], BF16, tag="pt")
                            nc.scalar.activation(pt2, psS, Exp)
                            m0 = 2 * ch - 4 * qg
                            if m0 >= 0:
                                nc.vector.tensor_mul(pt2, pt2, masks[:, m0:m0 + 2, :])
                            for u in range(2):
                                kt = 2 * ch + u
                                nc.tensor.matmul(psO, lhsT=vt[:, kt, :],
                                                 rhs=pt2[:, u, :],
                                                 start=(kt == 0), stop=(kt == nkt - 1))
                            for u in range(2):
                                nc.tensor.matmul(
                                    psN, lhsT=ones_bf, rhs=pt2[:, u, :],
                                    start=(ch == 0 and u == 0),
                                    stop=(ch == nch - 1 and u == 1))
                        rcp = sml.tile([1, 512], F32, tag="rcp")
                        nc.vector.reciprocal(rcp, psN)
                        rdb = sml.tile([P, 512], F32, tag="rdb")
                        nc.gpsimd.partition_broadcast(rdb, rcp)
                        nc.vector.tensor_mul(
                            ot_sb[:, b * 2048 + qg * 512:b * 2048 + (qg + 1) * 512],
                            psO, rdb)
                # ship this head-pair's rows, overlap with next attention
                o_dst = SEND2[hl][:, :].rearrange("j (p c) -> p j c", p=P)
                nc.sync.dma_start(
                    out=o_dst, in_=ot_sb.rearrange("p (j c) -> p j c", j=NC_))
                _a2a(SEND2[hl][:, :], RECV2[hl][:, :])
            # output projection, emitted after both attention passes so the
            # in-order PE and DMA queues never block hl1's attention on
            # RECV2[0]
            otf_h = otf.rearrange("p (j two) c -> p two j c", two=2)
            nc.sync.dma_start(
                out=otf_h[:, 0, :, :],
                in_=RECV2[0][:, :].rearrange("j (p c) -> p j c", p=P))
            with tc.tile_wait_until(1.0):
              for rt in range(4):
                for ng in range(4):
                    ps = pp6.tile([P, 512], F32)
                    for i in range(8):
                        nc.tensor.matmul(
                            ps, lhsT=otf[:, 2 * i, rt * 128:(rt + 1) * 128],
                            rhs=wo_sb[:, 2 * i, ng * 512:(ng + 1) * 512],
                            start=(i == 0), stop=(i == 7))
                    nc.scalar.copy(acc6[:, rt, ng * 512:(ng + 1) * 512], ps)
        # stage6-hl1 in its own scope: attention psum pools are closed, so
        # the second half of the output projection gets deep psum rotation
        with ExitStack() as sctx:
            pp6b = sctx.enter_context(tc.tile_pool(name=f"ps6b{rep}", bufs=4, space="PSUM"))
            nc.sync.dma_start(
                out=otf_h[:, 1, :, :],
                in_=RECV2[1][:, :].rearrange("j (p c) -> p j c", p=P))
            with tc.tile_wait_until(1.1):
              for rt in range(4):
                for ng in range(4):
                    ps = pp6b.tile([P, 512], F32, tag="ps6b")
                    for i in range(8):
                        nc.tensor.matmul(
                            ps, lhsT=otf[:, 2 * i + 1, rt * 128:(rt + 1) * 128],
                            rhs=wo_sb[:, 2 * i + 1, ng * 512:(ng + 1) * 512],
                            start=(i == 0), stop=(i == 7))
                    nc.vector.tensor_add(
                        acc6[:, rt, ng * 512:(ng + 1) * 512],
                        acc6[:, rt, ng * 512:(ng + 1) * 512], ps)
            for rt in range(4):
                nc.sync.dma_start(out=OUT[rt * 128:(rt + 1) * 128, :],
                                  in_=acc6[:, rt, :])


# ---------------------------------------------------------------------------
# Host-side prep
# ---------------------------------------------------------------------------

def _bf(a):
    return np.asarray(a, dtype=np.float32).astype(BF16NP)


def _prep_weights(wq_a, q_norm_w, wq_b, wkv_a, kv_norm_w, wkv_b, wo,
                  freqs_cos, freqs_sin):
    wkv_a_lat = wkv_a[:, :KV_LORA]
    wkv_a_rope = wkv_a[:, KV_LORA:] * S4
    wkv_a_rope = np.concatenate([wkv_a_rope[:, 0::2], wkv_a_rope[:, 1::2]], axis=1)
    WAKVh = np.concatenate([wkv_a_lat, wkv_a_rope], axis=1)           # [2048, 576]

    wqb = (wq_b * S4) * q_norm_w[:, None]
    wqb = wqb.reshape(Q_LORA, NH, 192)
    nope_cols = wqb[:, :, :NOPE].reshape(Q_LORA, NH * NOPE)
    rope_e = wqb[:, :, NOPE + 0::2].reshape(Q_LORA, NH * 32)
    rope_o = wqb[:, :, NOPE + 1::2].reshape(Q_LORA, NH * 32)
    WQBh = np.concatenate([nope_cols, rope_e, rope_o], axis=1)        # [1536, 3072]

    WKVBh = (wkv_b * kv_norm_w[:, None]).reshape(KV_LORA, NH, 256).copy()
    WKVBh[:, :, :NOPE] *= S4                                          # K nope cols
    WKVBh = WKVBh.reshape(KV_LORA, NH * 256)
    pos = np.arange(R) % S
    COS = freqs_cos[pos].astype(np.float32)                           # [4096, 32]
    SIN = freqs_sin[pos].astype(np.float32)
    return dict(WAKV=_bf(WAKVh), WAQ=_bf(wq_a), WQB=_bf(WQBh), WKVB=_bf(WKVBh),
                WO=_bf(wo), COS=COS, SIN=SIN)


def _prep_in_maps(inputs):
    x = np.asarray(inputs["x"], dtype=np.float32).reshape(R, 2048)
    W = _prep_weights(
        np.asarray(inputs["wq_a"]), np.asarray(inputs["q_norm_w"]),
        np.asarray(inputs["wq_b"]), np.asarray(inputs["wkv_a"]),
        np.asarray(inputs["kv_norm_w"]), np.asarray(inputs["wkv_b"]),
        np.asarray(inputs["wo"]),
        np.asarray(inputs["freqs_cos"]), np.asarray(inputs["freqs_sin"]))
    in_maps = []
    for c in range(NC_):
        rows = slice(c * LR, (c + 1) * LR)
        in_maps.append({
            "xt": np.ascontiguousarray(x[rows].T).astype(BF16NP),
            "wakv": W["WAKV"], "waq": W["WAQ"], "wqb": W["WQB"],
            "wkvb": W["WKVB"], "wo": W["WO"],
            "cost": np.ascontiguousarray(np.tile(W["COS"][rows].T, (4, 1))),
            "sint": np.ascontiguousarray(np.tile(W["SIN"][rows].T, (4, 1))),
        })
    return in_maps


_NC_CACHE = []


def _get_nc():
    if not _NC_CACHE:
        _NC_CACHE.append(build_kernel())
    return _NC_CACHE[0]


def kernel(**inputs) -> np.ndarray:
    in_maps = _prep_in_maps(inputs)
    nc = _get_nc()
    res = run_bass_kernel_spmd(nc, in_maps, core_ids=list(range(NC_)))
    outs = [res.results[c]["out"] for c in range(NC_)]
    return np.concatenate(outs, axis=0).reshape(B, S, 2048).astype(np.float32)
